# revision 36
# baseline (speedup 1.0000x reference)
"""Trainium2 Bass/Tile kernel for nn_Attention_3418793967804.

8-way data parallel over batch (1 batch per NeuronCore). Per core:
qkv 1x1 conv (+folded BN), 4-head attention over 2304 positions,
depthwise 3x3 conv on v, residual add, final 1x1 conv (+folded BN).

Layout: S^T score tiles (keys on partitions) via row-packed K=32 bf16
matmuls; exp on the scalar engine PSUM->SBUF; attention-value matmuls
column-tiled (two heads per PSUM bank) plus a replicated-ones stationary
for the softmax denominators (column-pair overlap makes the extra
denominator pass nearly free on the PE); depthwise 3x3 conv computed on
the Vector engine as per-tap scalar_tensor_tensor chains over the padded
v image (frees ~28us of PE time); x/qkv weights in bf16 (halves the
input DMA); c2 chunks scheduled into freed PSUM slots at i-chunk
transitions; both conv biases folded off the PE.
"""
import numpy as np

import concourse.bass as bass
import concourse.mybir as mybir
import concourse.tile as tile
from concourse import bacc

F32 = mybir.dt.float32
F32R = mybir.dt.float32r
BF16 = mybir.dt.bfloat16
EXP = mybir.ActivationFunctionType.Exp
MULT = mybir.AluOpType.mult
ADD = mybir.AluOpType.add

CH = 256
HW = 2304
H = W = 48
NH = 4
DK = 32
DH = 64
SCALE = float(DK) ** -0.5
EPS = 1e-3

IC_SIZES = [512, 512, 512, 512, 256]
IC_STARTS = [0, 512, 1024, 1536, 2048]
JB = 18          # 2304 / 128 j-blocks
QN = 384         # qkv spatial chunk = 8 rows of 48
NQ = HW // QN    # 6
PW = 50          # padded width/height


def build_consts(qkv_w, qkv_g, qkv_b, qkv_m, qkv_v, c1_w, c1_g, c1_b, c1_m, c1_v,
                 c2_w, c2_g, c2_b, c2_m, c2_v):
    """Fold BN into conv weights and pack into device-layout numpy arrays."""
    import ml_dtypes
    f = np.float32
    bf = ml_dtypes.bfloat16
    sq = qkv_g / np.sqrt(qkv_v + EPS)
    Wq = (qkv_w[:, :, 0, 0] * sq[:, None]).astype(f)       # (512, 256)
    bq = (qkv_b - qkv_m * sq).astype(f)                    # (512,)
    s1 = c1_g / np.sqrt(c1_v + EPS)
    W1 = (c1_w[:, 0, :, :] * s1[:, None, None]).astype(f)  # (256, 3, 3)
    b1 = (c1_b - c1_m * s1).astype(f)
    s2 = c2_g / np.sqrt(c2_v + EPS)
    W2 = (c2_w[:, :, 0, 0] * s2[:, None]).astype(f)        # (256, 256)
    b2 = (c2_b - c2_m * s2).astype(f)

    # qkv output channel permutation: cols 0-127 Q_all (h*32+dk), 128-255 K_all,
    # 256-511 v in natural c = h*64+d order
    perm = np.zeros(512, dtype=np.int64)
    for col in range(128):
        h, dk = col // 32, col % 32
        perm[col] = 128 * h + dk
        perm[128 + col] = 128 * h + 32 + dk
    for col in range(256):
        h, d = col // 64, col % 64
        perm[256 + col] = 128 * h + 64 + d
    wt = np.ascontiguousarray(Wq[perm].T).astype(bf)       # (256 ic, 512 col)
    bqkv = np.zeros((128, 4), f)
    for occ in range(4):
        bqkv[:, occ] = bq[perm[occ * 128:(occ + 1) * 128]]

    # depthwise taps as per-partition scalars: w1t[p, c*9+tap]
    w1t = np.zeros((128, 18), f)
    for c in range(2):
        for tap in range(9):
            di, dj = tap // 3, tap % 3
            w1t[:, c * 9 + tap] = W1[c * 128:(c + 1) * 128, di, dj]

    import ml_dtypes as mld
    ident = np.eye(128, dtype=np.float32).astype(mld.bfloat16)
    w2t = np.ascontiguousarray(W2.T)                        # (256 c, 256 oc)
    # the dw bias passes linearly through the final conv: fold it there
    b2e = (b2 + W2 @ b1).astype(f)
    b2p = np.stack([b2e[0:128], b2e[128:256]], axis=1).astype(f)  # (128, 2)
    return dict(wt=wt, bqkv=bqkv, w1t=w1t, w2t=w2t, ident=ident, b2p=b2p)


def build_nc(debug=False):
    nc = bacc.Bacc("TRN2", target_bir_lowering=False, debug=False,
                   enable_asserts=True, num_devices=8)
    dp = {}
    def din(name, shape, dt=F32):
        dp[name] = nc.dram_tensor(name, list(shape), dt, kind="ExternalInput").ap()
    din("x", (256, HW), BF16)
    din("wt", (256, 512), BF16)
    din("bqkv", (128, 4))
    din("w1t", (128, 18))
    din("w2t", (256, 256), F32R)
    din("b2p", (128, 2))
    din("ident", (128, 128), BF16)
    out_d = nc.dram_tensor("out", [256, HW], F32, kind="ExternalOutput").ap()
    dbg = {}
    if debug:
        for name, shape in [("dq", (128, HW)), ("dk", (128, HW)),
                            ("dvt", (128, JB * 256)), ("ddw0", (128, HW)),
                            ("ddw1", (128, HW)), ("dy0", (128, HW)),
                            ("dy1", (128, HW)), ("dot0", (128, HW)), ("dot1", (128, HW))]:
            dbg[name] = nc.dram_tensor(name, list(shape), F32, kind="ExternalOutput").ap()

    with tile.TileContext(nc) as tc:
        build_body(nc, tc, dp, out_d, dbg)
    nc.compile()
    return nc


def build_body(nc, tc, dp, out_d, dbg):
    from contextlib import ExitStack
    with ExitStack() as ctx:
        ep = ctx.enter_context
        wpool = ep(tc.tile_pool(name="w", bufs=1))
        xpool = ep(tc.tile_pool(name="x", bufs=1))
        qkpool = ep(tc.tile_pool(name="qk", bufs=1))
        vtpool = ep(tc.tile_pool(name="vt", bufs=1))
        vppool = ep(tc.tile_pool(name="vp", bufs=1))
        dwpool = ep(tc.tile_pool(name="dw", bufs=1))
        ypool = ep(tc.tile_pool(name="y", bufs=1))
        ppool = ep(tc.tile_pool(name="pp", bufs=7))
        npool = ep(tc.tile_pool(name="np", bufs=2))
        otpool = ep(tc.tile_pool(name="ot", bufs=1))
        obpool = ep(tc.tile_pool(name="ob", bufs=3))

        # --- weights & inputs ---
        wt_r = [wpool.tile([128, 512], BF16, tag=f"wt{c}", name=f"wt{c}") for c in range(2)]
        w1t_r = wpool.tile([128, 18], F32, tag="w1t", name="w1t")
        w2t_r = [wpool.tile([128, 256], F32R, tag=f"w2t{c}", name=f"w2t{c}") for c in range(2)]
        bq_f = wpool.tile([128, 4], F32, tag="bqf", name="bqf")
        b2_f = wpool.tile([128, 2], F32, tag="b2f", name="b2f")
        x_r = [xpool.tile([128, HW], BF16, tag=f"x{c}", name=f"x{c}") for c in range(2)]

        # critical-path loads first: wt + bias + x quarters (c-interleaved)
        for c in range(2):
            nc.sync.dma_start(wt_r[c][:], dp["wt"][128 * c:128 * (c + 1), :])
        nc.sync.dma_start(bq_f[:], dp["bqkv"][:])
        nc.sync.dma_start(b2_f[:], dp["b2p"][:])
        nc.sync.dma_start(w1t_r[:], dp["w1t"][:])
        for qr in range(4):
            qsl = slice(qr * (HW // 4), (qr + 1) * (HW // 4))
            for c in range(2):
                nc.sync.dma_start(x_r[c][:, qsl], dp["x"][128 * c:128 * (c + 1), qsl])

        Q = qkpool.tile([128, HW], BF16, tag="Q", name="Q")
        K = qkpool.tile([128, HW], BF16, tag="K", name="K")
        VT = vtpool.tile([128, JB * 256], BF16, tag="VT", name="VT")
        ones_b = vtpool.tile([128, 64], BF16, tag="onesb", name="onesb")
        nc.gpsimd.memset(ones_b[:], 1.0)
        id_b = vtpool.tile([128, 128], BF16, tag="idb", name="idb")
        nc.sync.dma_start(id_b[:], dp["ident"][:])
        vp = [vppool.tile([128, PW * PW], F32, tag=f"vp{c}", name=f"vp{c}") for c in range(2)]
        vf = [vppool.tile([128, HW], BF16, tag=f"vf{c}", name=f"vf{c}") for c in range(2)]
        for c in range(2):
            nc.gpsimd.memset(vp[c][:], 0.0)
        dwacc = [dwpool.tile([128, HW], F32, tag=f"dwa{c}", name=f"dwa{c}") for c in range(2)]
        y_all = [ypool.tile([128, HW], F32, tag=f"y{c}", name=f"y{c}") for c in range(2)]
        ot = [otpool.tile([128, HW], F32R, tag=f"ot{c}", name=f"ot{c}") for c in range(2)]

        with tc.tile_pool(name="psS", bufs=2, space="PSUM") as psS, \
             tc.tile_pool(name="psU", bufs=1, space="PSUM") as psU:

            def emit_qkv(occ, g):
                # one 384-wide chunk of the qkv projection for output group occ
                ps = psS.tile([128, 1024], F32, tag="s2", name="s2")
                sl = slice(g * QN, (g + 1) * QN)
                for c in range(2):
                    nc.tensor.matmul(
                        ps[:, 0:QN], wt_r[c][:, occ * 128:(occ + 1) * 128],
                        x_r[c][:, sl], start=(c == 0), stop=(c == 1))
                bias_ap = bq_f[:, occ:occ + 1]
                if occ == 0:
                    nc.vector.tensor_scalar_add(Q[:, sl], ps[:, 0:QN], bias_ap)
                elif occ == 1:
                    nc.vector.tensor_scalar_add(K[:, sl], ps[:, 0:QN], bias_ap)
                else:
                    c = occ - 2
                    vp3 = vp[c][:].rearrange("p (r w) -> p r w", w=PW)
                    dst = vp3[:, 1 + 8 * g:1 + 8 * g + 8, 1:49]
                    srcp = ps[:, 0:QN].rearrange("p (r w) -> p r w", w=48)
                    nc.vector.tensor_scalar_add(dst, srcp, bias_ap)
                    nc.vector.tensor_scalar_add(vf[c][:, sl], ps[:, 0:QN], bias_ap)

            def emit_dw_half(c, half):
                # depthwise 3x3 conv on the Vector engine: 9-tap
                # scalar_tensor_tensor accumulation chain over the padded
                # image, 24 output rows per call
                vp3 = vp[c][:].rearrange("p (r w) -> p r w", w=PW)
                dw3 = dwacc[c][:].rearrange("p (r w) -> p r w", w=48)
                r0 = 24 * half
                out3 = dw3[:, r0:r0 + 24, :]
                for tap in range(9):
                    di, dj = tap // 3, tap % 3
                    in0 = vp3[:, r0 + di:r0 + di + 24, dj:dj + 48]
                    wsc = w1t_r[:, c * 9 + tap:c * 9 + tap + 1]
                    if tap == 0:
                        nc.vector.tensor_scalar_mul(out3, in0, wsc)
                    else:
                        nc.vector.scalar_tensor_tensor(
                            out=out3, in0=in0, scalar=wsc, in1=out3,
                            op0=MULT, op1=ADD)

            def emit_vtT(jb):
                # V^T 128x128 block transposes on the PE (bf16, via identity)
                for c in range(2):
                    ps = psS.tile([128, 1024], F32, tag="s2", name="s2")
                    nc.tensor.transpose(ps[:, 0:64].bitcast(BF16),
                                        vf[c][:, jb * 128:(jb + 1) * 128], id_b[:])
                    nc.vector.tensor_copy(
                        VT[:, jb * 256 + 128 * c:jb * 256 + 128 * (c + 1)],
                        ps[:, 0:64].bitcast(BF16))

            def emit_ot(c, ic):
                n = IC_SIZES[ic]
                isl = slice(IC_STARTS[ic], IC_STARTS[ic] + n)
                nc.vector.tensor_add(ot[c][:, isl], dwacc[c][:, isl],
                                     y_all[c][:, isl])

            def emit_c2(occ, k, slot=None):
                n2 = IC_SIZES[k]
                isl2 = slice(IC_STARTS[k], IC_STARTS[k] + n2)
                ps = slot() if slot else psS.tile([128, 1024], F32, tag="s2", name="s2")
                for c in range(2):
                    nc.tensor.matmul(ps[:, 0:n2],
                                     w2t_r[c][:, occ * 128:(occ + 1) * 128],
                                     ot[c][:, isl2], start=(c == 0), stop=(c == 1))
                ob = obpool.tile([128, 512], F32, tag="ob", name="ob")
                nc.vector.tensor_scalar_add(ob[:, 0:n2], ps[:, 0:n2],
                                            b2_f[:, occ:occ + 1])
                nc.sync.dma_start(out_d[occ * 128:(occ + 1) * 128, isl2], ob[:, 0:n2])

            # minimal qkv pre-work: just what the first attention iterations
            # need; the rest interleaves into ic 0 via qkv_sched
            for occ, g in [(0, 0), (0, 1), (1, 0), (2, 0), (3, 0)]:
                emit_qkv(occ, g)
            qkv_sched = {
                0: [(1, 1)], 1: [(2, 1), (3, 1)], 2: [(0, 2)],
                3: [(1, 2)], 4: [(2, 2), (3, 2)], 5: [(0, 3)],
                6: [(1, 3)], 7: [(2, 3), (3, 3)], 8: [(0, 4)],
                9: [(1, 4)], 10: [(2, 4), (3, 4)], 11: [(0, 5)],
                12: [(1, 5)], 13: [(2, 5), (3, 5)],
            }
            # dw chains: (ic, jb) -> (c, half); half 0 needs vp chunks g<=3,
            # half 1 needs all chunks
            dw_sched = {(1, 1): (0, 0), (1, 7): (1, 0),
                        (1, 13): (0, 1), (2, 1): (1, 1)}

            # ic -> ot/c2 chunks emitted at the END of that ic (after its
            # normalization frees the U/D psum slots); ot(c,k) needs y(k) and
            # the dw half covering its columns; c2(k) needs ot-chunk k
            trans_sched = {
                1: [('ot', 0, 0), ('ot', 1, 0)],
                2: [('ot', 0, 1), ('ot', 1, 1), ('c2', 0, 0), ('c2', 1, 0)],
                3: [('ot', 0, 2), ('ot', 1, 2), ('c2', 0, 1), ('c2', 1, 1),
                    ('c2', 0, 2), ('c2', 1, 2)],
                4: [('ot', 0, 3), ('ot', 1, 3), ('ot', 0, 4), ('ot', 1, 4)],
            }
            for ic in range(5):
                n = IC_SIZES[ic]
                i0 = IC_STARTS[ic]
                isl = slice(i0, i0 + n)
                # U/D accumulators created lazily (at the first emit_av) so the
                # previous transition's aux chunks can take earlier buffer
                # versions of the same psum tags
                avt = {}

                def get_avt():
                    if not avt:
                        avt['U'] = [psU.tile([128, 512], F32, tag=f"Up{p}",
                                             name=f"Up{p}") for p in range(2)]
                        avt['D'] = [psU.tile([128, 512], F32, tag=f"Dp{p}",
                                             name=f"Dp{p}") for p in range(2)]
                    return avt

                def emit_qk_exp(jb):
                    jsl = slice(jb * 128, (jb + 1) * 128)
                    p2s = []
                    for hp in range(2):
                        s2 = psS.tile([128, 1024], F32, tag="s2", name="s2")
                        for hh in range(2):
                            h = 2 * hp + hh
                            nc.tensor.matmul(
                                s2[:, hh * 512:hh * 512 + n],
                                K[32 * h:32 * (h + 1), jsl],
                                Q[32 * h:32 * (h + 1), isl],
                                start=True, stop=True, tile_position=(32 * h, 0))
                        p2 = ppool.tile([128, 1024], BF16, tag="p2", name="p2")
                        if n == 512:
                            nc.scalar.activation(p2[:], s2[:], EXP, scale=SCALE)
                        else:
                            s3 = s2[:].rearrange("p (a b) -> p a b", b=512)[:, :, 0:n]
                            p3 = p2[:].rearrange("p (a b) -> p a b", b=512)[:, :, 0:n]
                            nc.scalar.activation(p3, s3, EXP, scale=SCALE)
                        p2s.append(p2)
                    return p2s

                def emit_av(jb, p2s):
                    # column-tiled (128x64): value matmul + replicated-denominator
                    # matmul per head; LDWEIGHTS of one tile overlaps the other.
                    t = get_avt()
                    for hp in range(2):
                        for hh in range(2):
                            h = 2 * hp + hh
                            mov = p2s[hp][:, hh * 512:hh * 512 + n]
                            nc.tensor.matmul(
                                t['U'][hp][64 * hh:64 * hh + 64, 0:n],
                                VT[:, jb * 256 + 64 * h:jb * 256 + 64 * (h + 1)],
                                mov, start=(jb == 0), stop=(jb == JB - 1),
                                tile_position=(0, 64 * hh))
                            nc.tensor.matmul(
                                t['D'][hp][64 * hh:64 * hh + 64, 0:n],
                                ones_b[:], mov,
                                start=(jb == 0), stop=(jb == JB - 1),
                                tile_position=(0, 64 * hh))

                def emit_trans(prev_ic):
                    # previous transition's ot/c2 chunks, emitted 1-2 jb into
                    # this ic so their DVE deps don't stall the in-order PE
                    # stream at the boundary; c2 takes pre-accumulator buffer
                    # versions of the U/D psum tags
                    slots = [lambda p=p: psU.tile([128, 512], F32, tag=f"Up{p}",
                                                  name=f"auxu{p}") for p in range(2)]
                    slots += [lambda p=p: psU.tile([128, 512], F32, tag=f"Dp{p}",
                                                   name=f"auxd{p}") for p in range(2)]
                    si = 0
                    for kind, a, b_ in trans_sched.get(prev_ic, []):
                        if kind == 'ot':
                            emit_ot(a, b_)
                        else:
                            emit_c2(a, b_, slot=slots[si % 4])
                            si += 1

                # 2-deep software pipeline: AV of jb-2 lands after QK+exp of jb
                pend = []
                for jb in range(JB):
                    pend.append(emit_qk_exp(jb))
                    if jb == 1 and ic >= 1:
                        emit_trans(ic - 1)
                    if jb >= 2:
                        emit_av(jb - 2, pend.pop(0))
                    if ic == 0:
                        for occ, g in qkv_sched.get(jb, ()):
                            emit_qkv(occ, g)
                        emit_vtT(jb)
                    if (ic, jb) in dw_sched:
                        c, half = dw_sched[(ic, jb)]
                        emit_dw_half(c, half)
                    if (ic, jb) == (0, 5):
                        # late-use weight loads, clear of the transpose burst
                        for c in range(2):
                            nc.sync.dma_start(w2t_r[c][:], dp["w2t"][128 * c:128 * (c + 1), :])
                emit_av(JB - 2, pend.pop(0))
                emit_av(JB - 1, pend.pop(0))

                # normalization: y = U * 1/den, all partition-aligned
                for hp in range(2):
                    rb = npool.tile([128, 512], F32, tag="rb", name="rb")
                    nc.vector.reciprocal_approx_fast(rb[:, 0:n], avt['D'][hp][:, 0:n])
                    nc.vector.tensor_mul(y_all[hp][:, isl], avt['U'][hp][:, 0:n],
                                         rb[:, 0:n])

            # --- tail: last transition's ot chunks + remaining c2 ---
            for kind, a, b_ in trans_sched.get(4, []):
                emit_ot(a, b_)
            for occ in range(2):
                for k in (3, 4):
                    emit_c2(occ, k)

        if dbg:
            nc.gpsimd.dma_start(dbg["dq"][:], Q[:])
            nc.gpsimd.dma_start(dbg["dk"][:], K[:])
            nc.gpsimd.dma_start(dbg["dvt"][:], VT[:])
            nc.sync.dma_start(dbg["ddw0"][:], dwacc[0][:])
            nc.sync.dma_start(dbg["ddw1"][:], dwacc[1][:])
            nc.sync.dma_start(dbg["dy0"][:], y_all[0][:])
            nc.sync.dma_start(dbg["dy1"][:], y_all[1][:])
            nc.sync.dma_start(dbg["dot0"][:], ot[0][:].bitcast(F32))
            nc.sync.dma_start(dbg["dot1"][:], ot[1][:].bitcast(F32))


def make_in_maps(x_full, consts):
    import ml_dtypes
    maps = []
    for b in range(8):
        m = dict(consts)
        m["x"] = np.ascontiguousarray(
            x_full[b].reshape(256, HW).astype(ml_dtypes.bfloat16))
        maps.append(m)
    return maps

_CACHED = {}


def _get_nc():
    if 'nc' not in _CACHED:
        _CACHED['nc'] = build_nc(debug=False)
    return _CACHED['nc']


def kernel(**inputs):
    """Full (unsharded) inputs -> full output (8, 256, 48, 48) float32."""
    from concourse.bass_utils import run_bass_kernel_spmd

    x = np.asarray(inputs['x'], dtype=np.float32)
    consts = build_consts(**{k: np.asarray(v) for k, v in inputs.items()
                             if k != 'x'})
    in_maps = make_in_maps(x, consts)
    nc = _get_nc()
    res = run_bass_kernel_spmd(nc, in_maps, list(range(8)))
    out = np.stack([res.results[b]['out'].reshape(256, 48, 48)
                    for b in range(8)])
    return out.astype(np.float32)


# revision 37
# speedup vs baseline: 1.0600x; 1.0600x over previous
"""Trainium2 Bass/Tile kernel for nn_Attention_3418793967804.

8-way data parallel over batch (1 batch per NeuronCore). Per core:
qkv 1x1 conv (+folded BN), 4-head attention over 2304 positions,
depthwise 3x3 conv on v, residual add, final 1x1 conv (+folded BN).

Layout: S^T score tiles (keys on partitions) via row-packed K=32 bf16
matmuls; exp on the scalar engine PSUM->SBUF; attention-value matmuls
column-tiled (two heads per PSUM bank) plus a replicated-ones stationary
for the softmax denominators (column-pair overlap makes the extra
denominator pass nearly free on the PE); depthwise 3x3 conv computed on
the Vector engine as per-tap scalar_tensor_tensor chains over the padded
v image (frees ~28us of PE time); x/qkv weights in bf16 (halves the
input DMA); c2 chunks scheduled into freed PSUM slots at i-chunk
transitions; both conv biases folded off the PE.
"""
import numpy as np

import concourse.bass as bass
import concourse.mybir as mybir
import concourse.tile as tile
from concourse import bacc

F32 = mybir.dt.float32
F32R = mybir.dt.float32r
BF16 = mybir.dt.bfloat16
EXP = mybir.ActivationFunctionType.Exp
MULT = mybir.AluOpType.mult
ADD = mybir.AluOpType.add

CH = 256
HW = 2304
H = W = 48
NH = 4
DK = 32
DH = 64
SCALE = float(DK) ** -0.5
EPS = 1e-3

IC_SIZES = [512, 512, 512, 512, 256]
IC_STARTS = [0, 512, 1024, 1536, 2048]
JB = 18          # 2304 / 128 j-blocks
QN = 384         # qkv spatial chunk = 8 rows of 48
NQ = HW // QN    # 6
PW = 50          # padded width/height


def build_consts(qkv_w, qkv_g, qkv_b, qkv_m, qkv_v, c1_w, c1_g, c1_b, c1_m, c1_v,
                 c2_w, c2_g, c2_b, c2_m, c2_v):
    """Fold BN into conv weights and pack into device-layout numpy arrays."""
    import ml_dtypes
    f = np.float32
    bf = ml_dtypes.bfloat16
    sq = qkv_g / np.sqrt(qkv_v + EPS)
    Wq = (qkv_w[:, :, 0, 0] * sq[:, None]).astype(f)       # (512, 256)
    bq = (qkv_b - qkv_m * sq).astype(f)                    # (512,)
    s1 = c1_g / np.sqrt(c1_v + EPS)
    W1 = (c1_w[:, 0, :, :] * s1[:, None, None]).astype(f)  # (256, 3, 3)
    b1 = (c1_b - c1_m * s1).astype(f)
    s2 = c2_g / np.sqrt(c2_v + EPS)
    W2 = (c2_w[:, :, 0, 0] * s2[:, None]).astype(f)        # (256, 256)
    b2 = (c2_b - c2_m * s2).astype(f)

    # qkv output channel permutation: cols 0-127 Q_all (h*32+dk), 128-255 K_all,
    # 256-511 v in natural c = h*64+d order
    perm = np.zeros(512, dtype=np.int64)
    for col in range(128):
        h, dk = col // 32, col % 32
        perm[col] = 128 * h + dk
        perm[128 + col] = 128 * h + 32 + dk
    for col in range(256):
        h, d = col // 64, col % 64
        perm[256 + col] = 128 * h + 64 + d
    wt = np.ascontiguousarray(Wq[perm].T).astype(bf)       # (256 ic, 512 col)
    bqkv = np.zeros((128, 4), f)
    for occ in range(4):
        bqkv[:, occ] = bq[perm[occ * 128:(occ + 1) * 128]]

    # depthwise taps as per-partition scalars: w1t[p, c*9+tap]
    w1t = np.zeros((128, 18), f)
    for c in range(2):
        for tap in range(9):
            di, dj = tap // 3, tap % 3
            w1t[:, c * 9 + tap] = W1[c * 128:(c + 1) * 128, di, dj]

    import ml_dtypes as mld
    ident = np.eye(128, dtype=np.float32).astype(mld.bfloat16)
    w2t = np.ascontiguousarray(W2.T)                        # (256 c, 256 oc)
    # the dw bias passes linearly through the final conv: fold it there
    b2e = (b2 + W2 @ b1).astype(f)
    b2p = np.stack([b2e[0:128], b2e[128:256]], axis=1).astype(f)  # (128, 2)
    return dict(wt=wt, bqkv=bqkv, w1t=w1t, w2t=w2t, ident=ident, b2p=b2p)


def build_nc(debug=False):
    nc = bacc.Bacc("TRN2", target_bir_lowering=False, debug=False,
                   enable_asserts=True, num_devices=8)
    dp = {}
    def din(name, shape, dt=F32):
        dp[name] = nc.dram_tensor(name, list(shape), dt, kind="ExternalInput").ap()
    din("x", (256, HW), BF16)
    din("wt", (256, 512), BF16)
    din("bqkv", (128, 4))
    din("w1t", (128, 18))
    din("w2t", (256, 256), F32R)
    din("b2p", (128, 2))
    din("ident", (128, 128), BF16)
    out_d = nc.dram_tensor("out", [256, HW], F32, kind="ExternalOutput").ap()
    dbg = {}
    if debug:
        for name, shape in [("dq", (128, HW)), ("dk", (128, HW)),
                            ("dvt", (128, JB * 256)), ("ddw0", (128, HW)),
                            ("ddw1", (128, HW)), ("dy0", (128, HW)),
                            ("dy1", (128, HW)), ("dot0", (128, HW)), ("dot1", (128, HW))]:
            dbg[name] = nc.dram_tensor(name, list(shape), F32, kind="ExternalOutput").ap()

    with tile.TileContext(nc) as tc:
        build_body(nc, tc, dp, out_d, dbg)
    nc.compile()
    return nc


def build_body(nc, tc, dp, out_d, dbg):
    from contextlib import ExitStack
    with ExitStack() as ctx:
        ep = ctx.enter_context
        wpool = ep(tc.tile_pool(name="w", bufs=1))
        xpool = ep(tc.tile_pool(name="x", bufs=1))
        qkpool = ep(tc.tile_pool(name="qk", bufs=1))
        vtpool = ep(tc.tile_pool(name="vt", bufs=1))
        vppool = ep(tc.tile_pool(name="vp", bufs=1))
        dwpool = ep(tc.tile_pool(name="dw", bufs=1))
        ypool = ep(tc.tile_pool(name="y", bufs=1))
        ppool = ep(tc.tile_pool(name="pp", bufs=7))
        npool = ep(tc.tile_pool(name="np", bufs=2))
        otpool = ep(tc.tile_pool(name="ot", bufs=1))
        obpool = ep(tc.tile_pool(name="ob", bufs=3))

        # --- weights & inputs ---
        wt_r = [wpool.tile([128, 512], BF16, tag=f"wt{c}", name=f"wt{c}") for c in range(2)]
        w1t_r = wpool.tile([128, 18], F32, tag="w1t", name="w1t")
        w2t_r = [wpool.tile([128, 256], F32R, tag=f"w2t{c}", name=f"w2t{c}") for c in range(2)]
        bq_f = wpool.tile([128, 4], F32, tag="bqf", name="bqf")
        b2_f = wpool.tile([128, 2], F32, tag="b2f", name="b2f")
        x_r = [xpool.tile([128, HW], BF16, tag=f"x{c}", name=f"x{c}") for c in range(2)]

        # critical-path loads first: wt + bias + x quarters (c-interleaved)
        for c in range(2):
            nc.sync.dma_start(wt_r[c][:], dp["wt"][128 * c:128 * (c + 1), :])
        nc.sync.dma_start(bq_f[:], dp["bqkv"][:])
        nc.sync.dma_start(b2_f[:], dp["b2p"][:])
        nc.sync.dma_start(w1t_r[:], dp["w1t"][:])
        for qr in range(4):
            qsl = slice(qr * (HW // 4), (qr + 1) * (HW // 4))
            for c in range(2):
                nc.sync.dma_start(x_r[c][:, qsl], dp["x"][128 * c:128 * (c + 1), qsl])

        Q = qkpool.tile([128, HW], BF16, tag="Q", name="Q")
        K = qkpool.tile([128, HW], BF16, tag="K", name="K")
        VT = vtpool.tile([128, JB * 256], BF16, tag="VT", name="VT")
        ones_b = vtpool.tile([128, 64], BF16, tag="onesb", name="onesb")
        nc.gpsimd.memset(ones_b[:], 1.0)
        id_b = vtpool.tile([128, 128], BF16, tag="idb", name="idb")
        nc.sync.dma_start(id_b[:], dp["ident"][:])
        vp = [vppool.tile([128, PW * PW], F32, tag=f"vp{c}", name=f"vp{c}") for c in range(2)]
        vf = [vppool.tile([128, HW], BF16, tag=f"vf{c}", name=f"vf{c}") for c in range(2)]
        for c in range(2):
            nc.gpsimd.memset(vp[c][:], 0.0)
        dwacc = [dwpool.tile([128, HW], F32, tag=f"dwa{c}", name=f"dwa{c}") for c in range(2)]
        y_all = [ypool.tile([128, HW], F32, tag=f"y{c}", name=f"y{c}") for c in range(2)]
        ot = [otpool.tile([128, HW], F32R, tag=f"ot{c}", name=f"ot{c}") for c in range(2)]

        with tc.tile_pool(name="psS", bufs=2, space="PSUM") as psS, \
             tc.tile_pool(name="psU", bufs=1, space="PSUM") as psU:

            def emit_qkv(occ, g):
                # one 384-wide chunk of the qkv projection for output group occ
                ps = psS.tile([128, 1024], F32, tag="s2", name="s2")
                sl = slice(g * QN, (g + 1) * QN)
                for c in range(2):
                    nc.tensor.matmul(
                        ps[:, 0:QN], wt_r[c][:, occ * 128:(occ + 1) * 128],
                        x_r[c][:, sl], start=(c == 0), stop=(c == 1))
                bias_ap = bq_f[:, occ:occ + 1]
                if occ == 0:
                    nc.vector.tensor_scalar_add(Q[:, sl], ps[:, 0:QN], bias_ap)
                elif occ == 1:
                    nc.vector.tensor_scalar_add(K[:, sl], ps[:, 0:QN], bias_ap)
                else:
                    c = occ - 2
                    vp3 = vp[c][:].rearrange("p (r w) -> p r w", w=PW)
                    dst = vp3[:, 1 + 8 * g:1 + 8 * g + 8, 1:49]
                    srcp = ps[:, 0:QN].rearrange("p (r w) -> p r w", w=48)
                    nc.vector.tensor_scalar_add(dst, srcp, bias_ap)
                    nc.vector.tensor_scalar_add(vf[c][:, sl], ps[:, 0:QN], bias_ap)

            def emit_dw_half(c, half):
                # depthwise 3x3 conv on the Vector engine: 9-tap
                # scalar_tensor_tensor accumulation chain over the padded
                # image, 24 output rows per call
                vp3 = vp[c][:].rearrange("p (r w) -> p r w", w=PW)
                dw3 = dwacc[c][:].rearrange("p (r w) -> p r w", w=48)
                r0 = 24 * half
                out3 = dw3[:, r0:r0 + 24, :]
                for tap in range(9):
                    di, dj = tap // 3, tap % 3
                    in0 = vp3[:, r0 + di:r0 + di + 24, dj:dj + 48]
                    wsc = w1t_r[:, c * 9 + tap:c * 9 + tap + 1]
                    if tap == 0:
                        nc.vector.tensor_scalar_mul(out3, in0, wsc)
                    else:
                        nc.vector.scalar_tensor_tensor(
                            out=out3, in0=in0, scalar=wsc, in1=out3,
                            op0=MULT, op1=ADD)

            def emit_vtT(jb):
                # V^T 128x128 block transposes on the PE (bf16, via identity)
                for c in range(2):
                    ps = psS.tile([128, 1024], F32, tag="s2", name="s2")
                    nc.tensor.transpose(ps[:, 0:64].bitcast(BF16),
                                        vf[c][:, jb * 128:(jb + 1) * 128], id_b[:])
                    nc.vector.tensor_copy(
                        VT[:, jb * 256 + 128 * c:jb * 256 + 128 * (c + 1)],
                        ps[:, 0:64].bitcast(BF16))

            def emit_ot(c, ic):
                n = IC_SIZES[ic]
                isl = slice(IC_STARTS[ic], IC_STARTS[ic] + n)
                nc.vector.tensor_add(ot[c][:, isl], dwacc[c][:, isl],
                                     y_all[c][:, isl])

            def emit_c2(occ, k, slot=None):
                n2 = IC_SIZES[k]
                isl2 = slice(IC_STARTS[k], IC_STARTS[k] + n2)
                ps = slot() if slot else psS.tile([128, 1024], F32, tag="s2", name="s2")
                for c in range(2):
                    nc.tensor.matmul(ps[:, 0:n2],
                                     w2t_r[c][:, occ * 128:(occ + 1) * 128],
                                     ot[c][:, isl2], start=(c == 0), stop=(c == 1))
                ob = obpool.tile([128, 512], F32, tag="ob", name="ob")
                nc.vector.tensor_scalar_add(ob[:, 0:n2], ps[:, 0:n2],
                                            b2_f[:, occ:occ + 1])
                nc.sync.dma_start(out_d[occ * 128:(occ + 1) * 128, isl2], ob[:, 0:n2])

            # minimal qkv pre-work: just what the first attention iterations
            # need; the rest interleaves into ic 0 via qkv_sched
            for occ, g in [(0, 0), (0, 1), (1, 0), (2, 0), (3, 0)]:
                emit_qkv(occ, g)
            qkv_sched = {
                0: [(1, 1)], 1: [(2, 1), (3, 1)], 2: [(0, 2)],
                3: [(1, 2)], 4: [(2, 2), (3, 2)], 5: [(0, 3)],
                6: [(1, 3)], 7: [(2, 3), (3, 3)], 8: [(0, 4)],
                9: [(1, 4)], 10: [(2, 4), (3, 4)], 11: [(0, 5)],
                12: [(1, 5)], 13: [(2, 5), (3, 5)],
            }
            # dw chains: (ic, jb) -> (c, half); half 0 needs vp chunks g<=3,
            # half 1 needs all chunks
            dw_sched = {(1, 1): (0, 0), (1, 7): (1, 0),
                        (1, 13): (0, 1), (2, 1): (1, 1)}

            # ic -> ot/c2 chunks emitted at the END of that ic (after its
            # normalization frees the U/D psum slots); ot(c,k) needs y(k) and
            # the dw half covering its columns; c2(k) needs ot-chunk k
            trans_sched = {
                1: [('ot', 0, 0), ('ot', 1, 0)],
                2: [('ot', 0, 1), ('ot', 1, 1), ('c2', 0, 0), ('c2', 1, 0)],
                3: [('ot', 0, 2), ('ot', 1, 2), ('c2', 0, 1), ('c2', 1, 1),
                    ('c2', 0, 2), ('c2', 1, 2)],
                4: [('ot', 0, 3), ('ot', 1, 3), ('ot', 0, 4), ('ot', 1, 4)],
            }
            for ic in range(5):
                n = IC_SIZES[ic]
                i0 = IC_STARTS[ic]
                isl = slice(i0, i0 + n)
                Up = [psU.tile([128, 512], F32, tag=f"Up{p}", name=f"Up{p}") for p in range(2)]
                Dp = [psU.tile([128, 512], F32, tag=f"Dp{p}", name=f"Dp{p}") for p in range(2)]

                def emit_qk_exp(jb):
                    jsl = slice(jb * 128, (jb + 1) * 128)
                    p2s = []
                    for hp in range(2):
                        s2 = psS.tile([128, 1024], F32, tag="s2", name="s2")
                        for hh in range(2):
                            h = 2 * hp + hh
                            nc.tensor.matmul(
                                s2[:, hh * 512:hh * 512 + n],
                                K[32 * h:32 * (h + 1), jsl],
                                Q[32 * h:32 * (h + 1), isl],
                                start=True, stop=True, tile_position=(32 * h, 0))
                        p2 = ppool.tile([128, 1024], BF16, tag="p2", name="p2")
                        if n == 512:
                            nc.scalar.activation(p2[:], s2[:], EXP, scale=SCALE)
                        else:
                            s3 = s2[:].rearrange("p (a b) -> p a b", b=512)[:, :, 0:n]
                            p3 = p2[:].rearrange("p (a b) -> p a b", b=512)[:, :, 0:n]
                            nc.scalar.activation(p3, s3, EXP, scale=SCALE)
                        p2s.append(p2)
                    return p2s

                def emit_av(jb, p2s):
                    # column-tiled (128x64): value matmul + replicated-denominator
                    # matmul per head; LDWEIGHTS of one tile overlaps the other.
                    for hp in range(2):
                        for hh in range(2):
                            h = 2 * hp + hh
                            mov = p2s[hp][:, hh * 512:hh * 512 + n]
                            nc.tensor.matmul(
                                Up[hp][64 * hh:64 * hh + 64, 0:n],
                                VT[:, jb * 256 + 64 * h:jb * 256 + 64 * (h + 1)],
                                mov, start=(jb == 0), stop=(jb == JB - 1),
                                tile_position=(0, 64 * hh))
                            nc.tensor.matmul(
                                Dp[hp][64 * hh:64 * hh + 64, 0:n],
                                ones_b[:], mov,
                                start=(jb == 0), stop=(jb == JB - 1),
                                tile_position=(0, 64 * hh))

                # 1-deep software pipeline: AV of jb-1 lands after QK+exp of jb
                prev = None
                for jb in range(JB):
                    p2s = emit_qk_exp(jb)
                    if prev is not None:
                        emit_av(jb - 1, prev)
                    prev = p2s
                    if ic == 0:
                        for occ, g in qkv_sched.get(jb, ()):
                            emit_qkv(occ, g)
                        emit_vtT(jb)
                    if (ic, jb) in dw_sched:
                        c, half = dw_sched[(ic, jb)]
                        emit_dw_half(c, half)
                    if (ic, jb) == (0, 5):
                        # late-use weight loads, clear of the transpose burst
                        for c in range(2):
                            nc.sync.dma_start(w2t_r[c][:], dp["w2t"][128 * c:128 * (c + 1), :])
                emit_av(JB - 1, prev)

                # normalization: y = U * 1/den, all partition-aligned
                for hp in range(2):
                    rb = npool.tile([128, 512], F32, tag="rb", name="rb")
                    nc.vector.reciprocal_approx_fast(rb[:, 0:n], Dp[hp][:, 0:n])
                    nc.vector.tensor_mul(y_all[hp][:, isl], Up[hp][:, 0:n], rb[:, 0:n])

                # ot/c2 chunks; c2 rides the just-freed U/D psum slots so it
                # stays out of the S-tile rotation (no ACT starvation)
                slots = [lambda p=p: psU.tile([128, 512], F32, tag=f"Up{p}",
                                              name=f"auxu{p}") for p in range(2)]
                slots += [lambda p=p: psU.tile([128, 512], F32, tag=f"Dp{p}",
                                               name=f"auxd{p}") for p in range(2)]
                si = 0
                for kind, a, b_ in trans_sched.get(ic, []):
                    if kind == 'ot':
                        emit_ot(a, b_)
                    else:
                        emit_c2(a, b_, slot=slots[si % 4])
                        si += 1

            # --- tail: remaining c2 chunks ---
            for occ in range(2):
                for k in (3, 4):
                    emit_c2(occ, k)

        if dbg:
            nc.gpsimd.dma_start(dbg["dq"][:], Q[:])
            nc.gpsimd.dma_start(dbg["dk"][:], K[:])
            nc.gpsimd.dma_start(dbg["dvt"][:], VT[:])
            nc.sync.dma_start(dbg["ddw0"][:], dwacc[0][:])
            nc.sync.dma_start(dbg["ddw1"][:], dwacc[1][:])
            nc.sync.dma_start(dbg["dy0"][:], y_all[0][:])
            nc.sync.dma_start(dbg["dy1"][:], y_all[1][:])
            nc.sync.dma_start(dbg["dot0"][:], ot[0][:].bitcast(F32))
            nc.sync.dma_start(dbg["dot1"][:], ot[1][:].bitcast(F32))


def make_in_maps(x_full, consts):
    import ml_dtypes
    maps = []
    for b in range(8):
        m = dict(consts)
        m["x"] = np.ascontiguousarray(
            x_full[b].reshape(256, HW).astype(ml_dtypes.bfloat16))
        maps.append(m)
    return maps

_CACHED = {}


def _get_nc():
    if 'nc' not in _CACHED:
        _CACHED['nc'] = build_nc(debug=False)
    return _CACHED['nc']


def kernel(**inputs):
    """Full (unsharded) inputs -> full output (8, 256, 48, 48) float32."""
    from concourse.bass_utils import run_bass_kernel_spmd

    x = np.asarray(inputs['x'], dtype=np.float32)
    consts = build_consts(**{k: np.asarray(v) for k, v in inputs.items()
                             if k != 'x'})
    in_maps = make_in_maps(x, consts)
    nc = _get_nc()
    res = run_bass_kernel_spmd(nc, in_maps, list(range(8)))
    out = np.stack([res.results[b]['out'].reshape(256, 48, 48)
                    for b in range(8)])
    return out.astype(np.float32)
